# revision 37
# baseline (speedup 1.0000x reference)
"""Trainium2 Bass kernel: 3x3 same-padding Conv2D, NCHW.

Input  (16, 64, 128, 128) f32, weights (128, 64, 3, 3) OIHW, bias (128,).
Output (16, 128, 128, 128) f32.  8 NeuronCores, 2 images per core.

Strategy (image-pair packing, fp16 I/O):
  - The two images of a core share the 128 SBUF partitions: partitions
    0-63 hold img0's 64 input channels (zero-padded to 130x130),
    partitions 64-127 hold img1's.
  - Every conv tap (kh, kw) is a K=64 matmul; the img0 tap (partitions
    0-63, PSUM bank A) and img1 tap (partitions 64-127, bank B) are
    issued adjacently so the PE runs them concurrently on disjoint
    row-group halves -> 1 effective slot per tap, the K=128 ideal
    (9 N=512 slots per 8 output rows of both images).
  - Slabs: 15x8 rows, then 4+2+2 rows last so the final bias-add and
    store are tiny and the teardown barrier is reached ASAP.  8-row
    slab = 4 PSUM banks double-buffered across slabs; the 2-row slabs
    accumulate in the psA1/psB1 tag slots freed ~2us earlier.  Slab 0
    runs group-major (pair 0's nine taps need only x rows 0-5); later
    slabs tap-major.
  - Startup is input-bound: during the 8-core startup crunch each
    HWDGE ring moves ~1 packet (one partition's slice) per DMA engine
    per ~220ns, so a [128, *] DMA costs ~1.8us nearly independent of
    bytes, and the two HW rings (sync, scalar) progress in parallel.
    Critical transfers go one per ring, whole: wb (all taps, one DMA)
    first on sync, x rows 0-6 first on scalar.  First real matmul
    ~11.0-11.3us -- the two-ring packet-round floor.
  - Epilogue: ScalarE and VectorE each bias-add two banks into an fp16
    tile laid out [r, img, w]; one contiguous 512 KB store per 8-row
    slab on the scalar HWDGE ring; 4/2-row slab stores ride the
    otherwise-idle sync ring.  The final 2-row slab does one short
    bias-add per engine, then stores split by PARTITION halves across
    both rings (64 descriptors each = half a DMA-engine round; the
    last store's transfer + HBM receipt gate the teardown).
    Output DRAM layout is [cout, h, img, w]; the host transposes to
    [img, cout, h, w] and upcasts to f32 (tolerance is 2e-2; fp16
    output rounding is ~5e-4).
  - 39 short junk matmuls on a zeroed scratch tile (memset on the
    early-exiting GpSimd engine) keep the PE busy from ~6.4us until
    the first input lands (~10.6-11.2us).  The HAM activity monitor
    needs ~3.4us of GAPLESS PE activity to un-throttle the clock from
    1.2 to 2.4 GHz, and an idle gap before that restarts the wait, so
    the junk count errs long enough to cover the data-arrival jitter.
  - After bacc compile, two post-passes edit the BIR in place:
    _dedup_ldweights() strips InstLdweights that reload the AP already
    resident in the same PE-array half (the tap-major order loads each
    tap's lo/hi twice per 8-row slab), and _hoist_startup_dmas() moves
    the critical input DMAs plus the warm-up block into the entry
    block ahead of the all-engine barrier, so the HWDGE doorbells ring
    ~1.4us earlier and the PE warm-up starts at ~6.4us instead of
    ~7.3us.

Every instruction may carry at most ONE semaphore wait on this
toolchain -- bacc.Bacc's compile() pipeline enforces that, which is why
this builds a Bacc, not a raw bass.Bass.
"""

import sys

if "/opt/trn_rl_repo" not in sys.path:
    sys.path.insert(0, "/opt/trn_rl_repo")

import numpy as np

N_CORES = 8
IMGS_PER_CORE = 2
H = 128
W = 128
CIN = 64
COUT = 128
WPAD = W + 2  # 130: one zero column each side
HPAD = H + 2  # 130 rows (pad row above and below)
ROWS_PER_BANK = 4   # 4*128 = 512 f32 = one PSUM bank
ROWS_PER_SLAB = 8   # 2 banks per image, 4 banks per slab
N_TAPS = 9
N_JUNK = 36

_cache = {}


def _build_nc():
    import concourse.mybir as mybir
    from concourse import bacc
    from concourse.tile import TileContext

    f32 = mybir.dt.float32
    f16 = mybir.dt.float16

    nc = bacc.Bacc(target_bir_lowering=False)
    # partitions 0-63: img0 padded channels; 64-127: img1
    x_d = nc.dram_tensor("x", [128, HPAD * WPAD], f16, kind="ExternalInput")
    # w[tap] duplicated on both partition halves: wb[p, t*128+co]
    wb_d = nc.dram_tensor("wb", [128, N_TAPS * COUT], f16, kind="ExternalInput")
    b_d = nc.dram_tensor("b", [COUT, 1], f32, kind="ExternalInput")
    # [cout, h, img, w] fp16; host transposes to [img, cout, h, w] + f32
    out_d = nc.dram_tensor(
        "out", [COUT, H * IMGS_PER_CORE * W], f16, kind="ExternalOutput"
    )

    with TileContext(nc) as tc:
        with (
            tc.tile_pool(name="wpool", bufs=1) as wpool,
            tc.tile_pool(name="xpool", bufs=1) as xpool,
            tc.tile_pool(name="opool", bufs=5) as opool,
            tc.tile_pool(name="pspool", bufs=2, space="PSUM") as pspool,
        ):
            wb_sb = wpool.tile([128, N_TAPS * COUT], f16)
            b_f32 = wpool.tile([COUT, 1], f32)
            b_sb = b_f32[:]

            X = xpool.tile([128, HPAD * WPAD], f16)

            # During the 8-core startup crunch each HWDGE ring delivers
            # ~1 packet (= 1 partition's slice) per DMA engine per
            # ~220ns, i.e. ~1.8us per [128, *] DMA instruction,
            # near-independent of byte count -- and the two HW rings
            # (sync, scalar) progress in parallel.  So the critical
            # transfers go one per ring, whole: wb (all taps, one DMA)
            # first on sync, x rows 0-6 first on scalar; later x chunks
            # are merged into as few DMAs as the streaming schedule
            # allows to minimize packet rounds.
            junk_src = wpool.tile([128, COUT], f16)
            nc.gpsimd.memset(junk_src[:], 0)
            # wb is split by partition halves across both rings (each
            # half is a 4-packet round instead of 8), and x rows 0-6
            # rides the earlier-waking sync ring, so the first real
            # matmul's inputs all land by ~10.3us.  x rows 6-17 (needed
            # by slab 0's second bank pair at ~12.4us) is also
            # partition-split so each half fires a round earlier than a
            # full-width DMA would.
            nc.sync.dma_start(out=wb_sb[0:64, :], in_=wb_d[0:64, :])
            nc.scalar.dma_start(out=wb_sb[64:128, :], in_=wb_d[64:128, :])
            nc.sync.dma_start(out=X[:, 0 : 6 * WPAD], in_=x_d[:, 0 : 6 * WPAD])
            nc.scalar.dma_start(
                out=X[64:128, 6 * WPAD : 18 * WPAD],
                in_=x_d[64:128, 6 * WPAD : 18 * WPAD],
            )
            nc.sync.dma_start(
                out=X[0:64, 6 * WPAD : 18 * WPAD], in_=x_d[0:64, 6 * WPAD : 18 * WPAD]
            )
            # remaining rows stream in few/large chunks to keep the
            # completion sems ahead of the PE
            for r0, r1 in [(18, 30), (30, 62), (62, 94), (94, HPAD)]:
                nc.sync.dma_start(
                    out=X[:, r0 * WPAD : r1 * WPAD],
                    in_=x_d[:, r0 * WPAD : r1 * WPAD],
                )
            nc.scalar.dma_start(out=b_f32[:], in_=b_d[:])
            X3 = X.rearrange("p (r c) -> p r c", c=WPAD)

            # HAM warm-up: junk matmuls on a memset scratch tile bridge
            # the PE from the engine-sync preamble (~7.3us) to the first
            # input chunk landing (~11.2us), so the activity monitor
            # un-throttles the PE clock (1.2 -> 2.4 GHz) at ~10.7us.
            # Results land in a PSUM bank that slab 0 later overwrites
            # with start=True.
            warm = pspool.tile([COUT, ROWS_PER_BANK * W], f32, tag="psA0")
            for _ in range(N_JUNK):
                nc.tensor.matmul(
                    warm[:, 0:COUT],
                    junk_src[:],
                    junk_src[:],
                    start=True,
                    stop=True,
                )

            # 8-row slabs, then a 4-row and two 2-row slabs at the end:
            # short final bias-adds and a tiny final store reach the
            # teardown barrier ASAP.  The 2-row slabs accumulate in the
            # psA1/psB1 tag slots, whose buffers were freed by slabs
            # 13/14 ~2us before they are needed -- using psA0/psB0
            # would stall the PE on slab 15's bias-add.
            slabs = [(8 * s, 8) for s in range(15)] + [(120, 4), (124, 2), (126, 2)]
            for si, (h0, nrows) in enumerate(slabs):
                h1 = h0 + ROWS_PER_BANK
                bank_rows = min(nrows, ROWS_PER_BANK)
                tagA, tagB = ("psA1", "psB1") if nrows == 2 else ("psA0", "psB0")
                psA0 = pspool.tile([COUT, bank_rows * W], f32, tag=tagA)
                psB0 = pspool.tile([COUT, bank_rows * W], f32, tag=tagB)
                if nrows == 8:
                    psA1 = pspool.tile([COUT, ROWS_PER_BANK * W], f32, tag="psA1")
                    psB1 = pspool.tile([COUT, ROWS_PER_BANK * W], f32, tag="psB1")
                    pairs = [(psA0, psB0, h0), (psA1, psB1, h1)]
                else:
                    pairs = [(psA0, psB0, h0)]
                # slab 0 runs group-major: its first bank pair's nine
                # taps read only input rows 0-5 (the first sync chunk),
                # so compute starts before rows 6-9 land
                order = (
                    [(p, t) for p in pairs for t in range(N_TAPS)]
                    if si == 0
                    else [(p, t) for t in range(N_TAPS) for p in pairs]
                )
                for (psA, psB, h), t in order:
                    kh, kw = divmod(t, 3)
                    lo = wb_sb[0:CIN, t * COUT : (t + 1) * COUT]
                    hi = wb_sb[CIN:128, t * COUT : (t + 1) * COUT]
                    st = t == 0
                    sp = t == N_TAPS - 1
                    # adjacent lo/hi matmuls run concurrently on
                    # disjoint PE row-group halves (different banks)
                    nc.tensor.matmul(
                        psA[:],
                        lo,
                        X3[0:CIN, h + kh : h + kh + bank_rows, kw : kw + W],
                        start=st,
                        stop=sp,
                    )
                    nc.tensor.matmul(
                        psB[:],
                        hi,
                        X3[CIN:128, h + kh : h + kh + bank_rows, kw : kw + W],
                        start=st,
                        stop=sp,
                    )
                # bias-add into fp16 tile, layout [r, img(2), w(128)]
                ob = opool.tile([COUT, nrows * IMGS_PER_CORE * W], f16)
                obv = ob.rearrange("p (r i c) -> p r i c", i=IMGS_PER_CORE, c=W)
                psA0v = psA0.rearrange("p (r c) -> p r c", c=W)
                psB0v = psB0.rearrange("p (r c) -> p r c", c=W)
                out_col = h0 * IMGS_PER_CORE * W
                span = nrows * IMGS_PER_CORE * W
                if nrows == 8:
                    psA1v = psA1.rearrange("p (r c) -> p r c", c=W)
                    psB1v = psB1.rearrange("p (r c) -> p r c", c=W)
                    nc.scalar.add(obv[:, 0:4, 0, :], psA0v[:], b_sb)
                    nc.scalar.add(obv[:, 0:4, 1, :], psB0v[:], b_sb)
                    nc.vector.tensor_scalar_add(obv[:, 4:8, 0, :], psA1v[:], b_sb)
                    nc.vector.tensor_scalar_add(obv[:, 4:8, 1, :], psB1v[:], b_sb)
                    # one contiguous 512 KB store per slab
                    nc.scalar.dma_start(
                        out=out_d[:, out_col : out_col + span],
                        in_=ob[:, 0:span],
                    )
                elif si < len(slabs) - 1:
                    # 4/2-row slab: both engines evacuate in parallel
                    # (different banks); store on the otherwise-idle
                    # sync ring to keep the scalar ring drained for the
                    # final store
                    nc.scalar.add(obv[:, 0:nrows, 0, :], psA0v[:], b_sb)
                    nc.vector.tensor_scalar_add(obv[:, 0:nrows, 1, :], psB0v[:], b_sb)
                    nc.sync.dma_start(
                        out=out_d[:, out_col : out_col + span],
                        in_=ob[:, 0:span],
                    )
                else:
                    # final 2-row slab: one short bias-add per engine,
                    # then the store split by PARTITION halves across
                    # both rings -- 64 descriptors each is half a
                    # DMA-engine round, so the last store's transfer +
                    # HBM receipt (which gate the teardown) finish
                    # ~1us earlier than a 128-partition store would.
                    nc.scalar.add(obv[:, 0:nrows, 0, :], psA0v[:], b_sb)
                    nc.vector.tensor_scalar_add(obv[:, 0:nrows, 1, :], psB0v[:], b_sb)
                    nc.sync.dma_start(
                        out=out_d[0:64, out_col : out_col + span],
                        in_=ob[0:64, 0:span],
                    )
                    nc.scalar.dma_start(
                        out=out_d[64:128, out_col : out_col + span],
                        in_=ob[64:128, 0:span],
                    )
    nc.compile()
    _dedup_ldweights(nc)
    _hoist_startup_dmas(nc)
    return nc


def _hoist_startup_dmas(nc):
    """Move the two startup-critical DMA instructions (wb on sync,
    x rows 0-6 on scalar) from the tile block into the entry block,
    ahead of the all-engine barrier.

    The issuing engines are idle before the barrier (~6.4us) while the
    barrier itself only completes ~7.0us, so hoisting rings the HWDGE
    doorbells ~0.6us earlier and the rings wake sooner.  The hoisted
    DMAs carry no semaphore waits and their completion increments fire
    wherever the instruction lives, so the tile-side consumers are
    unaffected.
    """
    blocks = list(nc.main_func.blocks)
    main = next(bb for bb in blocks if bb.name == "main")
    tile_bb = next(
        bb
        for bb in blocks
        if "tile_context" in bb.name and not bb.name.endswith("_end")
    )
    tinsts = tile_bb.instructions
    limits = {"Activation": 2, "SP": 3}
    hoist = {"Activation": [], "SP": []}
    for inst in tinsts:
        eng = str(inst.engine).split(".")[-1]
        if (
            type(inst).__name__ == "InstDMACopy"
            and eng in hoist
            and len(hoist[eng]) < limits[eng]
        ):
            si = inst.sync_info
            if si and si.on_wait:
                continue
            hoist[eng].append(inst)
    minsts = main.instructions
    for eng, insts in hoist.items():
        for inst in insts:
            tinsts.remove(inst)
        pos = next(
            k
            for k, mi in enumerate(minsts)
            if str(mi.engine).split(".")[-1] == eng
        )
        for off, inst in enumerate(insts):
            minsts.insert(pos + off, inst)

    # Also hoist the HAM warm-up (junk_src memset + junk ldweights/
    # matmuls) ahead of the barrier: the PE reaches the entry block at
    # ~5.9us, so the warm-up runs 6.1-9.8us and the activity monitor
    # un-throttles the clock at ~9.6us -- before the first input chunk
    # lands -- instead of the PE idling at the barrier until ~7.3us.
    # The junk matmuls' semaphore increments are positional-independent
    # (consumers wait on absolute values), and the junk PSUM bank is
    # overwritten by slab 0's start=True.
    pe_junk = []
    n_mm = 0
    for inst in tinsts:
        if str(inst.engine).split(".")[-1] != "PE":
            continue
        nm = type(inst).__name__
        if nm == "InstLdweights":
            pe_junk.append(inst)
        elif nm == "InstMatmult":
            pe_junk.append(inst)
            n_mm += 1
            if n_mm == N_JUNK:
                break
        else:
            break
    memset = None
    for inst in tinsts:
        if (
            type(inst).__name__ == "InstMemset"
            and str(inst.engine).split(".")[-1] == "Pool"
        ):
            memset = inst
            break
    if memset is not None and n_mm == N_JUNK:
        tinsts.remove(memset)
        for k, mi in enumerate(minsts):
            if (
                str(mi.engine).split(".")[-1] == "Pool"
                and type(mi).__name__ == "InstDrain"
            ):
                minsts.insert(k, memset)
                break
        for inst in pe_junk:
            tinsts.remove(inst)
        pos = next(
            k
            for k, mi in enumerate(minsts)
            if str(mi.engine).split(".")[-1] == "PE"
        )
        for off, inst in enumerate(pe_junk):
            minsts.insert(pos + off, inst)


def _dedup_ldweights(nc):
    """Drop InstLdweights that reload the exact weights AP already
    resident in the same PE-array half.

    The tap-major slab order issues, per tap, the lo weights for bank
    pair 0 and again for bank pair 1 (same SBUF slice -> same array
    rows); the legalizer emits a fresh InstLdweights for every matmul.
    The second load is a no-op on the array state, but costs an NX
    issue slot (~3ns each, ~0.8us over the kernel).  LDW loads
    alternate between the two array halves (lo = rows 0-63, hi =
    64-127) and a load into one half does not disturb the other, so an
    LDW whose AP equals the AP of the LDW two back in the PE stream is
    reloading exactly what is still resident.  Only waitless/updateless
    LDWs are removed so the semaphore schedule is untouched.
    """
    for bb in nc.main_func.blocks:
        if "tile_context" not in bb.name or bb.name.endswith("_end"):
            continue
        insts = bb.instructions
        keys = []
        to_remove = []
        for inst in insts:
            if type(inst).__name__ != "InstLdweights":
                continue
            si = inst.sync_info
            has_sync = bool(si and (si.on_wait or si.on_update))
            key = str(inst.ins[0])
            if len(keys) >= 2 and keys[-2] == key and not has_sync:
                to_remove.append(inst)
            keys.append(key)
        for inst in to_remove:
            insts.remove(inst)


def _get_nc():
    if "nc" not in _cache:
        _cache["nc"] = _build_nc()
    return _cache["nc"]


def _prepare_in_maps(input_tensor, weights, bias):
    input_tensor = np.asarray(input_tensor, dtype=np.float32)
    weights = np.asarray(weights, dtype=np.float32)
    bias = np.asarray(bias, dtype=np.float32)
    # wb[ci, t*128+co] = W[co, ci, kh, kw], t = kh*3+kw; both halves
    w9 = weights.transpose(1, 2, 3, 0).reshape(CIN, N_TAPS * COUT)  # ci,(kh kw co)
    wb = np.empty((128, N_TAPS * COUT), dtype=np.float16)
    wb[0:CIN] = w9
    wb[CIN:128] = w9
    wb = np.ascontiguousarray(wb)
    b = np.ascontiguousarray(bias.reshape(COUT, 1))
    in_maps = []
    for c in range(N_CORES):
        imgs = input_tensor[c * IMGS_PER_CORE : (c + 1) * IMGS_PER_CORE]
        zp = np.zeros((IMGS_PER_CORE, CIN, HPAD, WPAD), dtype=np.float16)
        zp[:, :, 1 : H + 1, 1 : W + 1] = imgs
        shard = np.ascontiguousarray(zp.reshape(128, HPAD * WPAD))
        in_maps.append({"x": shard, "wb": wb, "b": b})
    return in_maps


def _gather(results):
    outs = []
    for c in range(N_CORES):
        o = results[c]["out"].reshape(COUT, H, IMGS_PER_CORE, W)
        outs.append(np.ascontiguousarray(o.transpose(2, 0, 1, 3), dtype=np.float32))
    return np.concatenate(outs, axis=0)


def kernel(input_tensor, weights, bias):
    from concourse.bass_utils import run_bass_kernel_spmd

    nc = _get_nc()
    in_maps = _prepare_in_maps(input_tensor, weights, bias)
    res = run_bass_kernel_spmd(nc, in_maps, core_ids=list(range(N_CORES)))
    return _gather(res.results)


# revision 40
# speedup vs baseline: 1.0242x; 1.0242x over previous
"""Trainium2 Bass kernel: 3x3 same-padding Conv2D, NCHW.

Input  (16, 64, 128, 128) f32, weights (128, 64, 3, 3) OIHW, bias (128,).
Output (16, 128, 128, 128) f32.  8 NeuronCores, 2 images per core.

Strategy (image-pair packing, fp16 I/O):
  - The two images of a core share the 128 SBUF partitions: partitions
    0-63 hold img0's 64 input channels (zero-padded to 130x130),
    partitions 64-127 hold img1's.
  - Every conv tap (kh, kw) is a K=64 matmul; the img0 tap (partitions
    0-63, PSUM bank A) and img1 tap (partitions 64-127, bank B) are
    issued adjacently so the PE runs them concurrently on disjoint
    row-group halves -> 1 effective slot per tap, the K=128 ideal
    (9 N=512 slots per 8 output rows of both images).
  - Slabs: 15x8 rows, then 4+2+2 rows last so the final bias-add and
    store are tiny and the teardown barrier is reached ASAP.  8-row
    slab = 4 PSUM banks double-buffered across slabs; the 2-row slabs
    accumulate in the psA1/psB1 tag slots freed ~2us earlier.  Slab 0
    runs group-major (pair 0's nine taps need only x rows 0-5); later
    slabs tap-major.
  - Startup is input-bound: during the 8-core startup crunch each
    HWDGE ring moves ~1 packet (one partition's slice) per DMA engine
    per ~220ns, so a [128, *] DMA costs ~1.8us nearly independent of
    bytes, and the two HW rings (sync, scalar) progress in parallel.
    Critical transfers go one per ring, whole: wb (all taps, one DMA)
    first on sync, x rows 0-6 first on scalar.  First real matmul
    ~11.0-11.3us -- the two-ring packet-round floor.
  - Epilogue: ScalarE and VectorE each bias-add two banks into an fp16
    tile laid out [r, img, w]; one contiguous 512 KB store per 8-row
    slab on the scalar HWDGE ring; 4/2-row slab stores ride the
    otherwise-idle sync ring.  The final 2-row slab does one short
    bias-add per engine, then stores split by PARTITION halves across
    both rings (64 descriptors each = half a DMA-engine round; the
    last store's transfer + HBM receipt gate the teardown).
    Output DRAM layout is [cout, h, img, w]; the host transposes to
    [img, cout, h, w] and upcasts to f32 (tolerance is 2e-2; fp16
    output rounding is ~5e-4).
  - 39 short junk matmuls on a zeroed scratch tile (memset on the
    early-exiting GpSimd engine) keep the PE busy from ~6.4us until
    the first input lands (~10.6-11.2us).  The HAM activity monitor
    needs ~3.4us of GAPLESS PE activity to un-throttle the clock from
    1.2 to 2.4 GHz, and an idle gap before that restarts the wait, so
    the junk count errs long enough to cover the data-arrival jitter.
  - After bacc compile, two post-passes edit the BIR in place:
    _dedup_ldweights() strips InstLdweights that reload the AP already
    resident in the same PE-array half (the tap-major order loads each
    tap's lo/hi twice per 8-row slab), and _hoist_startup_dmas() moves
    the critical input DMAs plus the warm-up block into the entry
    block ahead of the all-engine barrier, so the HWDGE doorbells ring
    ~1.4us earlier and the PE warm-up starts at ~6.4us instead of
    ~7.3us.

Every instruction may carry at most ONE semaphore wait on this
toolchain -- bacc.Bacc's compile() pipeline enforces that, which is why
this builds a Bacc, not a raw bass.Bass.
"""

import sys

if "/opt/trn_rl_repo" not in sys.path:
    sys.path.insert(0, "/opt/trn_rl_repo")

import numpy as np

N_CORES = 8
IMGS_PER_CORE = 2
H = 128
W = 128
CIN = 64
COUT = 128
WPAD = W + 2  # 130: one zero column each side
HPAD = H + 2  # 130 rows (pad row above and below)
ROWS_PER_BANK = 4   # 4*128 = 512 f32 = one PSUM bank
ROWS_PER_SLAB = 8   # 2 banks per image, 4 banks per slab
N_TAPS = 9
N_JUNK = 36

_cache = {}


def _build_nc():
    import concourse.mybir as mybir
    from concourse import bacc
    from concourse.tile import TileContext

    f32 = mybir.dt.float32
    f16 = mybir.dt.float16

    nc = bacc.Bacc(target_bir_lowering=False)
    # partitions 0-63: img0 padded channels; 64-127: img1
    x_d = nc.dram_tensor("x", [128, HPAD * WPAD], f16, kind="ExternalInput")
    # w[tap] duplicated on both partition halves: wb[p, t*128+co]
    wb_d = nc.dram_tensor("wb", [128, N_TAPS * COUT], f16, kind="ExternalInput")
    b_d = nc.dram_tensor("b", [COUT, 1], f32, kind="ExternalInput")
    # [cout, h, img, w] fp16; host transposes to [img, cout, h, w] + f32
    out_d = nc.dram_tensor(
        "out", [COUT, H * IMGS_PER_CORE * W], f16, kind="ExternalOutput"
    )

    with TileContext(nc) as tc:
        with (
            tc.tile_pool(name="wpool", bufs=1) as wpool,
            tc.tile_pool(name="xpool", bufs=1) as xpool,
            tc.tile_pool(name="opool", bufs=5) as opool,
            tc.tile_pool(name="pspool", bufs=2, space="PSUM") as pspool,
        ):
            wb_sb = wpool.tile([128, N_TAPS * COUT], f16)
            b_f32 = wpool.tile([COUT, 1], f32)
            b_sb = b_f32[:]

            X = xpool.tile([128, HPAD * WPAD], f16)

            # During the 8-core startup crunch each HWDGE ring delivers
            # ~1 packet (= 1 partition's slice) per DMA engine per
            # ~220ns, i.e. ~1.8us per [128, *] DMA instruction,
            # near-independent of byte count -- and the two HW rings
            # (sync, scalar) progress in parallel.  So the critical
            # transfers go one per ring, whole: wb (all taps, one DMA)
            # first on sync, x rows 0-6 first on scalar; later x chunks
            # are merged into as few DMAs as the streaming schedule
            # allows to minimize packet rounds.
            junk_src = wpool.tile([128, COUT], f16)
            nc.gpsimd.memset(junk_src[:], 0)
            # The critical transfers go one per ring, whole: wb (8
            # packets/engine) on sync, x rows 0-6 on scalar -- the
            # balanced 8+8 split of the 16 critical packet rounds.
            # x rows 6-17 (needed by slab 0's second bank pair at
            # ~12.8us) is partition-split across both rings so each
            # half is a half round and fires ~12.5us.
            nc.sync.dma_start(out=wb_sb[:], in_=wb_d[:])
            nc.scalar.dma_start(out=X[:, 0 : 6 * WPAD], in_=x_d[:, 0 : 6 * WPAD])
            nc.sync.dma_start(
                out=X[0:64, 6 * WPAD : 18 * WPAD], in_=x_d[0:64, 6 * WPAD : 18 * WPAD]
            )
            nc.scalar.dma_start(
                out=X[64:128, 6 * WPAD : 18 * WPAD],
                in_=x_d[64:128, 6 * WPAD : 18 * WPAD],
            )
            # remaining rows stream in few/large chunks to keep the
            # completion sems ahead of the PE
            for r0, r1 in [(18, 30), (30, 62), (62, 94), (94, HPAD)]:
                nc.sync.dma_start(
                    out=X[:, r0 * WPAD : r1 * WPAD],
                    in_=x_d[:, r0 * WPAD : r1 * WPAD],
                )
            nc.scalar.dma_start(out=b_f32[:], in_=b_d[:])
            X3 = X.rearrange("p (r c) -> p r c", c=WPAD)

            # HAM warm-up: junk matmuls on a memset scratch tile bridge
            # the PE from the engine-sync preamble (~7.3us) to the first
            # input chunk landing (~11.2us), so the activity monitor
            # un-throttles the PE clock (1.2 -> 2.4 GHz) at ~10.7us.
            # Results land in a PSUM bank that slab 0 later overwrites
            # with start=True.
            warm = pspool.tile([COUT, ROWS_PER_BANK * W], f32, tag="psA0")
            for _ in range(N_JUNK):
                nc.tensor.matmul(
                    warm[:, 0:COUT],
                    junk_src[:],
                    junk_src[:],
                    start=True,
                    stop=True,
                )

            # 8-row slabs, then a 4-row and two 2-row slabs at the end:
            # short final bias-adds and a tiny final store reach the
            # teardown barrier ASAP.  The 2-row slabs accumulate in the
            # psA1/psB1 tag slots, whose buffers were freed by slabs
            # 13/14 ~2us before they are needed -- using psA0/psB0
            # would stall the PE on slab 15's bias-add.
            slabs = [(8 * s, 8) for s in range(15)] + [(120, 4), (124, 2), (126, 2)]
            for si, (h0, nrows) in enumerate(slabs):
                h1 = h0 + ROWS_PER_BANK
                bank_rows = min(nrows, ROWS_PER_BANK)
                tagA, tagB = ("psA1", "psB1") if nrows == 2 else ("psA0", "psB0")
                psA0 = pspool.tile([COUT, bank_rows * W], f32, tag=tagA)
                psB0 = pspool.tile([COUT, bank_rows * W], f32, tag=tagB)
                if nrows == 8:
                    psA1 = pspool.tile([COUT, ROWS_PER_BANK * W], f32, tag="psA1")
                    psB1 = pspool.tile([COUT, ROWS_PER_BANK * W], f32, tag="psB1")
                    pairs = [(psA0, psB0, h0), (psA1, psB1, h1)]
                else:
                    pairs = [(psA0, psB0, h0)]
                # slab 0 runs group-major: its first bank pair's nine
                # taps read only input rows 0-5 (the first sync chunk),
                # so compute starts before rows 6-9 land
                order = (
                    [(p, t) for p in pairs for t in range(N_TAPS)]
                    if si == 0
                    else [(p, t) for t in range(N_TAPS) for p in pairs]
                )
                for (psA, psB, h), t in order:
                    kh, kw = divmod(t, 3)
                    lo = wb_sb[0:CIN, t * COUT : (t + 1) * COUT]
                    hi = wb_sb[CIN:128, t * COUT : (t + 1) * COUT]
                    st = t == 0
                    sp = t == N_TAPS - 1
                    # adjacent lo/hi matmuls run concurrently on
                    # disjoint PE row-group halves (different banks)
                    nc.tensor.matmul(
                        psA[:],
                        lo,
                        X3[0:CIN, h + kh : h + kh + bank_rows, kw : kw + W],
                        start=st,
                        stop=sp,
                    )
                    nc.tensor.matmul(
                        psB[:],
                        hi,
                        X3[CIN:128, h + kh : h + kh + bank_rows, kw : kw + W],
                        start=st,
                        stop=sp,
                    )
                # bias-add into fp16 tile, layout [r, img(2), w(128)]
                ob = opool.tile([COUT, nrows * IMGS_PER_CORE * W], f16)
                obv = ob.rearrange("p (r i c) -> p r i c", i=IMGS_PER_CORE, c=W)
                psA0v = psA0.rearrange("p (r c) -> p r c", c=W)
                psB0v = psB0.rearrange("p (r c) -> p r c", c=W)
                out_col = h0 * IMGS_PER_CORE * W
                span = nrows * IMGS_PER_CORE * W
                if nrows == 8:
                    psA1v = psA1.rearrange("p (r c) -> p r c", c=W)
                    psB1v = psB1.rearrange("p (r c) -> p r c", c=W)
                    nc.scalar.add(obv[:, 0:4, 0, :], psA0v[:], b_sb)
                    nc.scalar.add(obv[:, 0:4, 1, :], psB0v[:], b_sb)
                    nc.vector.tensor_scalar_add(obv[:, 4:8, 0, :], psA1v[:], b_sb)
                    nc.vector.tensor_scalar_add(obv[:, 4:8, 1, :], psB1v[:], b_sb)
                    # one contiguous 512 KB store per slab
                    nc.scalar.dma_start(
                        out=out_d[:, out_col : out_col + span],
                        in_=ob[:, 0:span],
                    )
                elif si < len(slabs) - 1:
                    # 4/2-row slab: both engines evacuate in parallel
                    # (different banks); store on the otherwise-idle
                    # sync ring to keep the scalar ring drained for the
                    # final store
                    nc.scalar.add(obv[:, 0:nrows, 0, :], psA0v[:], b_sb)
                    nc.vector.tensor_scalar_add(obv[:, 0:nrows, 1, :], psB0v[:], b_sb)
                    nc.sync.dma_start(
                        out=out_d[:, out_col : out_col + span],
                        in_=ob[:, 0:span],
                    )
                else:
                    # final 2-row slab: one short bias-add per engine,
                    # then the store split by PARTITION halves across
                    # both rings -- 64 descriptors each is half a
                    # DMA-engine round, so the last store's transfer +
                    # HBM receipt (which gate the teardown) finish
                    # ~1us earlier than a 128-partition store would.
                    nc.scalar.add(obv[:, 0:nrows, 0, :], psA0v[:], b_sb)
                    nc.vector.tensor_scalar_add(obv[:, 0:nrows, 1, :], psB0v[:], b_sb)
                    nc.sync.dma_start(
                        out=out_d[0:64, out_col : out_col + span],
                        in_=ob[0:64, 0:span],
                    )
                    nc.scalar.dma_start(
                        out=out_d[64:128, out_col : out_col + span],
                        in_=ob[64:128, 0:span],
                    )
    nc.compile()
    _dedup_ldweights(nc)
    _hoist_startup_dmas(nc)
    return nc


def _hoist_startup_dmas(nc):
    """Move the two startup-critical DMA instructions (wb on sync,
    x rows 0-6 on scalar) from the tile block into the entry block,
    ahead of the all-engine barrier.

    The issuing engines are idle before the barrier (~6.4us) while the
    barrier itself only completes ~7.0us, so hoisting rings the HWDGE
    doorbells ~0.6us earlier and the rings wake sooner.  The hoisted
    DMAs carry no semaphore waits and their completion increments fire
    wherever the instruction lives, so the tile-side consumers are
    unaffected.
    """
    blocks = list(nc.main_func.blocks)
    main = next(bb for bb in blocks if bb.name == "main")
    tile_bb = next(
        bb
        for bb in blocks
        if "tile_context" in bb.name and not bb.name.endswith("_end")
    )
    tinsts = tile_bb.instructions
    limits = {"Activation": 2, "SP": 2}
    hoist = {"Activation": [], "SP": []}
    for inst in tinsts:
        eng = str(inst.engine).split(".")[-1]
        if (
            type(inst).__name__ == "InstDMACopy"
            and eng in hoist
            and len(hoist[eng]) < limits[eng]
        ):
            si = inst.sync_info
            if si and si.on_wait:
                continue
            hoist[eng].append(inst)
    minsts = main.instructions
    for eng, insts in hoist.items():
        for inst in insts:
            tinsts.remove(inst)
        pos = next(
            k
            for k, mi in enumerate(minsts)
            if str(mi.engine).split(".")[-1] == eng
        )
        for off, inst in enumerate(insts):
            minsts.insert(pos + off, inst)

    # The HAM warm-up (junk matmuls) intentionally stays in the tile
    # block: hoisting it pre-barrier was tried and regressed -- the
    # busy PE delays the barrier (starving the later input chunks) and
    # the junk-to-data seam gap then resets the free-running HAM
    # window on ~half the draws, restarting the clock throttle.  With
    # the junk post-barrier it always abuts the data arrival.


def _dedup_ldweights(nc):
    """Drop InstLdweights that reload the exact weights AP already
    resident in the same PE-array half.

    The tap-major slab order issues, per tap, the lo weights for bank
    pair 0 and again for bank pair 1 (same SBUF slice -> same array
    rows); the legalizer emits a fresh InstLdweights for every matmul.
    The second load is a no-op on the array state, but costs an NX
    issue slot (~3ns each, ~0.8us over the kernel).  LDW loads
    alternate between the two array halves (lo = rows 0-63, hi =
    64-127) and a load into one half does not disturb the other, so an
    LDW whose AP equals the AP of the LDW two back in the PE stream is
    reloading exactly what is still resident.  Only waitless/updateless
    LDWs are removed so the semaphore schedule is untouched.
    """
    for bb in nc.main_func.blocks:
        if "tile_context" not in bb.name or bb.name.endswith("_end"):
            continue
        insts = bb.instructions
        keys = []
        to_remove = []
        for inst in insts:
            if type(inst).__name__ != "InstLdweights":
                continue
            si = inst.sync_info
            has_sync = bool(si and (si.on_wait or si.on_update))
            key = str(inst.ins[0])
            if len(keys) >= 2 and keys[-2] == key and not has_sync:
                to_remove.append(inst)
            keys.append(key)
        for inst in to_remove:
            insts.remove(inst)


def _get_nc():
    if "nc" not in _cache:
        _cache["nc"] = _build_nc()
    return _cache["nc"]


def _prepare_in_maps(input_tensor, weights, bias):
    input_tensor = np.asarray(input_tensor, dtype=np.float32)
    weights = np.asarray(weights, dtype=np.float32)
    bias = np.asarray(bias, dtype=np.float32)
    # wb[ci, t*128+co] = W[co, ci, kh, kw], t = kh*3+kw; both halves
    w9 = weights.transpose(1, 2, 3, 0).reshape(CIN, N_TAPS * COUT)  # ci,(kh kw co)
    wb = np.empty((128, N_TAPS * COUT), dtype=np.float16)
    wb[0:CIN] = w9
    wb[CIN:128] = w9
    wb = np.ascontiguousarray(wb)
    b = np.ascontiguousarray(bias.reshape(COUT, 1))
    in_maps = []
    for c in range(N_CORES):
        imgs = input_tensor[c * IMGS_PER_CORE : (c + 1) * IMGS_PER_CORE]
        zp = np.zeros((IMGS_PER_CORE, CIN, HPAD, WPAD), dtype=np.float16)
        zp[:, :, 1 : H + 1, 1 : W + 1] = imgs
        shard = np.ascontiguousarray(zp.reshape(128, HPAD * WPAD))
        in_maps.append({"x": shard, "wb": wb, "b": b})
    return in_maps


def _gather(results):
    outs = []
    for c in range(N_CORES):
        o = results[c]["out"].reshape(COUT, H, IMGS_PER_CORE, W)
        outs.append(np.ascontiguousarray(o.transpose(2, 0, 1, 3), dtype=np.float32))
    return np.concatenate(outs, axis=0)


def kernel(input_tensor, weights, bias):
    from concourse.bass_utils import run_bass_kernel_spmd

    nc = _get_nc()
    in_maps = _prepare_in_maps(input_tensor, weights, bias)
    res = run_bass_kernel_spmd(nc, in_maps, core_ids=list(range(N_CORES)))
    return _gather(res.results)


# revision 44
# speedup vs baseline: 1.0279x; 1.0037x over previous
"""Trainium2 Bass kernel: 3x3 same-padding Conv2D, NCHW.

Input  (16, 64, 128, 128) f32, weights (128, 64, 3, 3) OIHW, bias (128,).
Output (16, 128, 128, 128) f32.  8 NeuronCores, 2 images per core.

Strategy (image-pair packing, fp16 I/O):
  - The two images of a core share the 128 SBUF partitions: partitions
    0-63 hold img0's 64 input channels (zero-padded to 130x130),
    partitions 64-127 hold img1's.
  - Every conv tap (kh, kw) is a K=64 matmul; the img0 tap (partitions
    0-63, PSUM bank A) and img1 tap (partitions 64-127, bank B) are
    issued adjacently so the PE runs them concurrently on disjoint
    row-group halves -> 1 effective slot per tap, the K=128 ideal
    (9 N=512 slots per 8 output rows of both images).
  - Slabs: 15x8 rows, then 4+2+2 rows last so the final bias-add and
    store are tiny and the teardown barrier is reached ASAP.  8-row
    slab = 4 PSUM banks double-buffered across slabs; the 2-row slabs
    accumulate in the psA1/psB1 tag slots freed ~2us earlier.  Slab 0
    runs group-major (pair 0's nine taps need only x rows 0-5); later
    slabs tap-major.
  - Startup is input-bound: during the 8-core startup crunch each
    HWDGE ring moves ~1 packet (one partition's slice) per DMA engine
    per ~220-300ns, so a [128, *] DMA costs ~2us nearly independent
    of bytes, and the two HW rings (sync, scalar) progress in
    parallel.  Critical transfers go one per ring, whole: wb (all
    taps, one DMA) on sync, x rows 0-6 on scalar -- the balanced 8+8
    split of the 16 critical packet rounds.  First real matmul
    ~10.6-12us: the scalar ring's wake (~8.2us) plus its 8 rounds is
    the floor.
  - Epilogue: ScalarE and VectorE each bias-add two banks into an fp16
    tile laid out [r, img, w]; one contiguous 512 KB store per 8-row
    slab on the scalar HWDGE ring; 4/2-row slab stores ride the
    otherwise-idle sync ring.  The final 2-row slab does one short
    bias-add per engine, then stores split by PARTITION halves across
    both rings (64 descriptors each = half a DMA-engine round; the
    last store's transfer + HBM receipt gate the teardown).
    Output DRAM layout is [cout, h, img, w]; the host transposes to
    [img, cout, h, w] and upcasts to f32 (tolerance is 2e-2; fp16
    output rounding is ~5e-4).
  - 31 short junk matmuls on a zeroed scratch tile (memset on the
    early-exiting GpSimd engine) keep the PE busy from ~7.5us until
    the first input lands (~10.6-12us).  The HAM activity monitor
    needs ~3.4us of GAPLESS PE activity to un-throttle the clock from
    1.2 to 2.4 GHz, and an idle gap before that can restart the wait,
    so the junk count errs long enough to abut the data arrival.
  - After bacc compile, two post-passes edit the BIR in place:
    _dedup_ldweights() strips InstLdweights that reload the AP already
    resident in the same PE-array half (the tap-major order loads each
    tap's lo/hi twice per 8-row slab), and _hoist_startup_dmas() moves
    the four critical input DMAs into the entry block ahead of the
    all-engine barrier, so the HWDGE doorbells ring ~1.4us earlier and
    the rings wake before the tile body even starts.

Every instruction may carry at most ONE semaphore wait on this
toolchain -- bacc.Bacc's compile() pipeline enforces that, which is why
this builds a Bacc, not a raw bass.Bass.
"""

import sys

if "/opt/trn_rl_repo" not in sys.path:
    sys.path.insert(0, "/opt/trn_rl_repo")

import numpy as np

N_CORES = 8
IMGS_PER_CORE = 2
H = 128
W = 128
CIN = 64
COUT = 128
WPAD = W + 2  # 130: one zero column each side
HPAD = H + 2  # 130 rows (pad row above and below)
ROWS_PER_BANK = 4   # 4*128 = 512 f32 = one PSUM bank
ROWS_PER_SLAB = 8   # 2 banks per image, 4 banks per slab
N_TAPS = 9
N_JUNK = 36

_cache = {}


def _build_nc():
    import concourse.mybir as mybir
    from concourse import bacc
    from concourse.tile import TileContext

    f32 = mybir.dt.float32
    f16 = mybir.dt.float16

    nc = bacc.Bacc(target_bir_lowering=False)
    # partitions 0-63: img0 padded channels; 64-127: img1
    x_d = nc.dram_tensor("x", [128, HPAD * WPAD], f16, kind="ExternalInput")
    # w[tap] duplicated on both partition halves: wb[p, t*128+co]
    wb_d = nc.dram_tensor("wb", [128, N_TAPS * COUT], f16, kind="ExternalInput")
    b_d = nc.dram_tensor("b", [COUT, 1], f32, kind="ExternalInput")
    # [cout, h, img, w] fp16; host transposes to [img, cout, h, w] + f32
    out_d = nc.dram_tensor(
        "out", [COUT, H * IMGS_PER_CORE * W], f16, kind="ExternalOutput"
    )

    with TileContext(nc) as tc:
        with (
            tc.tile_pool(name="wpool", bufs=1) as wpool,
            tc.tile_pool(name="xpool", bufs=1) as xpool,
            tc.tile_pool(name="opool", bufs=5) as opool,
            tc.tile_pool(name="pspool", bufs=2, space="PSUM") as pspool,
        ):
            wb_sb = wpool.tile([128, N_TAPS * COUT], f16)
            b_f32 = wpool.tile([COUT, 1], f32)
            b_sb = b_f32[:]

            X = xpool.tile([128, HPAD * WPAD], f16)

            # During the 8-core startup crunch each HWDGE ring delivers
            # ~1 packet (= 1 partition's slice) per DMA engine per
            # ~220-300ns, i.e. ~2us per [128, *] DMA instruction,
            # near-independent of byte count -- and the two HW rings
            # (sync, scalar) progress in parallel.  Later x chunks are
            # merged into as few DMAs as the streaming schedule allows
            # to minimize packet rounds.
            junk_src = wpool.tile([128, COUT], f16)
            nc.gpsimd.memset(junk_src[:], 0)
            # The critical transfers go one per ring, whole: wb (8
            # packets/engine) on sync, x rows 0-6 on scalar -- the
            # balanced 8+8 split of the 16 critical packet rounds.
            # x rows 6-17 (needed by slab 0's second bank pair at
            # ~12.8us) is partition-split across both rings so each
            # half is a half round and fires ~12.5us.
            nc.sync.dma_start(out=wb_sb[:], in_=wb_d[:])
            nc.scalar.dma_start(out=X[:, 0 : 6 * WPAD], in_=x_d[:, 0 : 6 * WPAD])
            nc.sync.dma_start(
                out=X[0:64, 6 * WPAD : 18 * WPAD], in_=x_d[0:64, 6 * WPAD : 18 * WPAD]
            )
            nc.scalar.dma_start(
                out=X[64:128, 6 * WPAD : 18 * WPAD],
                in_=x_d[64:128, 6 * WPAD : 18 * WPAD],
            )
            # remaining rows stream in few/large chunks to keep the
            # completion sems ahead of the PE
            for r0, r1 in [(18, 30), (30, 62), (62, 94), (94, HPAD)]:
                nc.sync.dma_start(
                    out=X[:, r0 * WPAD : r1 * WPAD],
                    in_=x_d[:, r0 * WPAD : r1 * WPAD],
                )
            nc.scalar.dma_start(out=b_f32[:], in_=b_d[:])
            X3 = X.rearrange("p (r c) -> p r c", c=WPAD)

            # HAM warm-up: junk matmuls on a memset scratch tile bridge
            # the PE from the engine-sync preamble (~7.3us) to the first
            # input chunk landing (~11.2us), so the activity monitor
            # un-throttles the PE clock (1.2 -> 2.4 GHz) at ~10.7us.
            # Results land in a PSUM bank that slab 0 later overwrites
            # with start=True.
            warm = pspool.tile([COUT, ROWS_PER_BANK * W], f32, tag="psA0")
            for _ in range(N_JUNK):
                nc.tensor.matmul(
                    warm[:, 0:COUT],
                    junk_src[:],
                    junk_src[:],
                    start=True,
                    stop=True,
                )

            # 8-row slabs, then a 4-row and two 2-row slabs at the end:
            # short final bias-adds and a tiny final store reach the
            # teardown barrier ASAP.  The 2-row slabs accumulate in the
            # psA1/psB1 tag slots, whose buffers were freed by slabs
            # 13/14 ~2us before they are needed -- using psA0/psB0
            # would stall the PE on slab 15's bias-add.
            slabs = [(8 * s, 8) for s in range(15)] + [(120, 4), (124, 2), (126, 2)]
            for si, (h0, nrows) in enumerate(slabs):
                h1 = h0 + ROWS_PER_BANK
                bank_rows = min(nrows, ROWS_PER_BANK)
                tagA, tagB = ("psA1", "psB1") if nrows == 2 else ("psA0", "psB0")
                psA0 = pspool.tile([COUT, bank_rows * W], f32, tag=tagA)
                psB0 = pspool.tile([COUT, bank_rows * W], f32, tag=tagB)
                if nrows == 8:
                    psA1 = pspool.tile([COUT, ROWS_PER_BANK * W], f32, tag="psA1")
                    psB1 = pspool.tile([COUT, ROWS_PER_BANK * W], f32, tag="psB1")
                    pairs = [(psA0, psB0, h0), (psA1, psB1, h1)]
                else:
                    pairs = [(psA0, psB0, h0)]
                # slab 0 runs group-major: its first bank pair's nine
                # taps read only input rows 0-5 (the first sync chunk),
                # so compute starts before rows 6-9 land
                order = (
                    [(p, t) for p in pairs for t in range(N_TAPS)]
                    if si == 0
                    else [(p, t) for t in range(N_TAPS) for p in pairs]
                )
                for (psA, psB, h), t in order:
                    kh, kw = divmod(t, 3)
                    lo = wb_sb[0:CIN, t * COUT : (t + 1) * COUT]
                    hi = wb_sb[CIN:128, t * COUT : (t + 1) * COUT]
                    st = t == 0
                    sp = t == N_TAPS - 1
                    # adjacent lo/hi matmuls run concurrently on
                    # disjoint PE row-group halves (different banks)
                    nc.tensor.matmul(
                        psA[:],
                        lo,
                        X3[0:CIN, h + kh : h + kh + bank_rows, kw : kw + W],
                        start=st,
                        stop=sp,
                    )
                    nc.tensor.matmul(
                        psB[:],
                        hi,
                        X3[CIN:128, h + kh : h + kh + bank_rows, kw : kw + W],
                        start=st,
                        stop=sp,
                    )
                # bias-add into fp16 tile, layout [r, img(2), w(128)]
                ob = opool.tile([COUT, nrows * IMGS_PER_CORE * W], f16)
                obv = ob.rearrange("p (r i c) -> p r i c", i=IMGS_PER_CORE, c=W)
                psA0v = psA0.rearrange("p (r c) -> p r c", c=W)
                psB0v = psB0.rearrange("p (r c) -> p r c", c=W)
                out_col = h0 * IMGS_PER_CORE * W
                span = nrows * IMGS_PER_CORE * W
                if nrows == 8:
                    psA1v = psA1.rearrange("p (r c) -> p r c", c=W)
                    psB1v = psB1.rearrange("p (r c) -> p r c", c=W)
                    nc.scalar.add(obv[:, 0:4, 0, :], psA0v[:], b_sb)
                    nc.scalar.add(obv[:, 0:4, 1, :], psB0v[:], b_sb)
                    nc.vector.tensor_scalar_add(obv[:, 4:8, 0, :], psA1v[:], b_sb)
                    nc.vector.tensor_scalar_add(obv[:, 4:8, 1, :], psB1v[:], b_sb)
                    # one contiguous 512 KB store per slab
                    nc.scalar.dma_start(
                        out=out_d[:, out_col : out_col + span],
                        in_=ob[:, 0:span],
                    )
                elif si < len(slabs) - 1:
                    # 4/2-row slab: both engines evacuate in parallel
                    # (different banks); store on the otherwise-idle
                    # sync ring to keep the scalar ring drained for the
                    # final store
                    nc.scalar.add(obv[:, 0:nrows, 0, :], psA0v[:], b_sb)
                    nc.vector.tensor_scalar_add(obv[:, 0:nrows, 1, :], psB0v[:], b_sb)
                    nc.sync.dma_start(
                        out=out_d[:, out_col : out_col + span],
                        in_=ob[:, 0:span],
                    )
                else:
                    # final 2-row slab: one short bias-add per engine,
                    # then the store split by PARTITION halves across
                    # both rings -- 64 descriptors each is half a
                    # DMA-engine round, so the last store's transfer +
                    # HBM receipt (which gate the teardown) finish
                    # ~1us earlier than a 128-partition store would.
                    nc.scalar.add(obv[:, 0:nrows, 0, :], psA0v[:], b_sb)
                    nc.vector.tensor_scalar_add(obv[:, 0:nrows, 1, :], psB0v[:], b_sb)
                    nc.sync.dma_start(
                        out=out_d[0:64, out_col : out_col + span],
                        in_=ob[0:64, 0:span],
                    )
                    nc.scalar.dma_start(
                        out=out_d[64:128, out_col : out_col + span],
                        in_=ob[64:128, 0:span],
                    )
    nc.compile()
    _dedup_ldweights(nc)
    # NOTE: _hoist_startup_dmas (moving the critical input DMAs ahead
    # of the all-engine barrier) measured ~0.5-1us faster but produced
    # nonfinite output on ~half the runs once machine timing drifted --
    # the pre-barrier descriptors race per-core DGE initialization and
    # slab 0 then consumes garbage.  Correctness gates the speedup, so
    # the hoist is disabled.
    return nc


def _hoist_startup_dmas(nc):
    """Move the two startup-critical DMA instructions (wb on sync,
    x rows 0-6 on scalar) from the tile block into the entry block,
    ahead of the all-engine barrier.

    The issuing engines are idle before the barrier (~6.4us) while the
    barrier itself only completes ~7.0us, so hoisting rings the HWDGE
    doorbells ~0.6us earlier and the rings wake sooner.  The hoisted
    DMAs carry no semaphore waits and their completion increments fire
    wherever the instruction lives, so the tile-side consumers are
    unaffected.
    """
    blocks = list(nc.main_func.blocks)
    main = next(bb for bb in blocks if bb.name == "main")
    tile_bb = next(
        bb
        for bb in blocks
        if "tile_context" in bb.name and not bb.name.endswith("_end")
    )
    tinsts = tile_bb.instructions
    limits = {"Activation": 2, "SP": 2}
    hoist = {"Activation": [], "SP": []}
    for inst in tinsts:
        eng = str(inst.engine).split(".")[-1]
        if (
            type(inst).__name__ == "InstDMACopy"
            and eng in hoist
            and len(hoist[eng]) < limits[eng]
        ):
            si = inst.sync_info
            if si and si.on_wait:
                continue
            hoist[eng].append(inst)
    minsts = main.instructions
    for eng, insts in hoist.items():
        for inst in insts:
            tinsts.remove(inst)
        pos = next(
            k
            for k, mi in enumerate(minsts)
            if str(mi.engine).split(".")[-1] == eng
        )
        for off, inst in enumerate(insts):
            minsts.insert(pos + off, inst)

    # The HAM warm-up (junk matmuls) intentionally stays in the tile
    # block: hoisting it pre-barrier was tried and regressed -- the
    # busy PE delays the barrier (starving the later input chunks) and
    # the junk-to-data seam gap then resets the free-running HAM
    # window on ~half the draws, restarting the clock throttle.  With
    # the junk post-barrier it always abuts the data arrival.


def _dedup_ldweights(nc):
    """Drop InstLdweights that reload the exact weights AP already
    resident in the same PE-array half.

    The tap-major slab order issues, per tap, the lo weights for bank
    pair 0 and again for bank pair 1 (same SBUF slice -> same array
    rows); the legalizer emits a fresh InstLdweights for every matmul.
    The second load is a no-op on the array state, but costs an NX
    issue slot (~3ns each, ~0.8us over the kernel).  LDW loads
    alternate between the two array halves (lo = rows 0-63, hi =
    64-127) and a load into one half does not disturb the other, so an
    LDW whose AP equals the AP of the LDW two back in the PE stream is
    reloading exactly what is still resident.  Only waitless/updateless
    LDWs are removed so the semaphore schedule is untouched.
    """
    for bb in nc.main_func.blocks:
        if "tile_context" not in bb.name or bb.name.endswith("_end"):
            continue
        insts = bb.instructions
        keys = []
        to_remove = []
        for inst in insts:
            if type(inst).__name__ != "InstLdweights":
                continue
            si = inst.sync_info
            has_sync = bool(si and (si.on_wait or si.on_update))
            key = str(inst.ins[0])
            if len(keys) >= 2 and keys[-2] == key and not has_sync:
                to_remove.append(inst)
            keys.append(key)
        for inst in to_remove:
            insts.remove(inst)


def _get_nc():
    if "nc" not in _cache:
        _cache["nc"] = _build_nc()
    return _cache["nc"]


def _prepare_in_maps(input_tensor, weights, bias):
    input_tensor = np.asarray(input_tensor, dtype=np.float32)
    weights = np.asarray(weights, dtype=np.float32)
    bias = np.asarray(bias, dtype=np.float32)
    # wb[ci, t*128+co] = W[co, ci, kh, kw], t = kh*3+kw; both halves
    w9 = weights.transpose(1, 2, 3, 0).reshape(CIN, N_TAPS * COUT)  # ci,(kh kw co)
    wb = np.empty((128, N_TAPS * COUT), dtype=np.float16)
    wb[0:CIN] = w9
    wb[CIN:128] = w9
    wb = np.ascontiguousarray(wb)
    b = np.ascontiguousarray(bias.reshape(COUT, 1))
    in_maps = []
    for c in range(N_CORES):
        imgs = input_tensor[c * IMGS_PER_CORE : (c + 1) * IMGS_PER_CORE]
        zp = np.zeros((IMGS_PER_CORE, CIN, HPAD, WPAD), dtype=np.float16)
        zp[:, :, 1 : H + 1, 1 : W + 1] = imgs
        shard = np.ascontiguousarray(zp.reshape(128, HPAD * WPAD))
        in_maps.append({"x": shard, "wb": wb, "b": b})
    return in_maps


def _gather(results):
    outs = []
    for c in range(N_CORES):
        o = results[c]["out"].reshape(COUT, H, IMGS_PER_CORE, W)
        outs.append(np.ascontiguousarray(o.transpose(2, 0, 1, 3), dtype=np.float32))
    return np.concatenate(outs, axis=0)


def kernel(input_tensor, weights, bias):
    from concourse.bass_utils import run_bass_kernel_spmd

    nc = _get_nc()
    in_maps = _prepare_in_maps(input_tensor, weights, bias)
    res = run_bass_kernel_spmd(nc, in_maps, core_ids=list(range(N_CORES)))
    return _gather(res.results)


# revision 46
# speedup vs baseline: 1.0401x; 1.0118x over previous
"""Trainium2 Bass kernel: 3x3 same-padding Conv2D, NCHW.

Input  (16, 64, 128, 128) f32, weights (128, 64, 3, 3) OIHW, bias (128,).
Output (16, 128, 128, 128) f32.  8 NeuronCores, 2 images per core.

Strategy (image-pair packing, fp16 I/O):
  - The two images of a core share the 128 SBUF partitions: partitions
    0-63 hold img0's 64 input channels (zero-padded to 130x130),
    partitions 64-127 hold img1's.
  - Every conv tap (kh, kw) is a K=64 matmul; the img0 tap (partitions
    0-63, PSUM bank A) and img1 tap (partitions 64-127, bank B) are
    issued adjacently so the PE runs them concurrently on disjoint
    row-group halves -> 1 effective slot per tap, the K=128 ideal
    (9 N=512 slots per 8 output rows of both images).
  - Slabs: 15x8 rows, then 4+2+2 rows last so the final bias-add and
    store are tiny and the teardown barrier is reached ASAP.  8-row
    slab = 4 PSUM banks double-buffered across slabs; the 2-row slabs
    accumulate in the psA1/psB1 tag slots freed ~2us earlier.  Slab 0
    runs group-major (pair 0's nine taps need only x rows 0-5); later
    slabs tap-major.
  - Startup is input-bound: during the 8-core startup crunch each
    HWDGE ring moves ~1 packet (one partition's slice) per DMA engine
    per ~220-300ns, so a [128, *] DMA costs ~2us nearly independent
    of bytes, and the two HW rings (sync, scalar) progress in
    parallel.  Critical transfers go one per ring, whole: wb (all
    taps, one DMA) on sync, x rows 0-6 on scalar -- the balanced 8+8
    split of the 16 critical packet rounds.  First real matmul
    ~10.6-12us: the scalar ring's wake (~8.2us) plus its 8 rounds is
    the floor.
  - Epilogue: ScalarE and VectorE each bias-add two banks into an fp16
    tile laid out [r, img, w]; one contiguous 512 KB store per 8-row
    slab on the scalar HWDGE ring; 4/2-row slab stores ride the
    otherwise-idle sync ring.  The final 2-row slab does one short
    bias-add per engine, then stores split by PARTITION halves across
    both rings (64 descriptors each = half a DMA-engine round; the
    last store's transfer + HBM receipt gate the teardown).
    Output DRAM layout is [cout, h, img, w]; the host transposes to
    [img, cout, h, w] and upcasts to f32 (tolerance is 2e-2; fp16
    output rounding is ~5e-4).
  - 36 short junk matmuls on a zeroed scratch tile (memset on the
    early-exiting GpSimd engine) keep the PE busy from ~7.5us until
    the first input lands (~10.7-13us).  The HAM activity monitor
    needs ~3.4us of GAPLESS PE activity to un-throttle the clock from
    1.2 to 2.4 GHz, and an idle gap before that can restart the wait,
    so the junk count errs long enough to abut the data arrival.
  - After bacc compile, _dedup_ldweights() edits the BIR in place to
    strip InstLdweights that reload the AP already resident in the
    same PE-array half (the tap-major order loads each tap's lo/hi
    twice per 8-row slab).  A second pass that hoisted the critical
    input DMAs ahead of the all-engine barrier (~0.5-1us faster) is
    DISABLED: it intermittently raced per-core DGE initialization and
    produced nonfinite output in slab 0.

Every instruction may carry at most ONE semaphore wait on this
toolchain -- bacc.Bacc's compile() pipeline enforces that, which is why
this builds a Bacc, not a raw bass.Bass.
"""

import sys

if "/opt/trn_rl_repo" not in sys.path:
    sys.path.insert(0, "/opt/trn_rl_repo")

import numpy as np

N_CORES = 8
IMGS_PER_CORE = 2
H = 128
W = 128
CIN = 64
COUT = 128
WPAD = W + 2  # 130: one zero column each side
HPAD = H + 2  # 130 rows (pad row above and below)
ROWS_PER_BANK = 4   # 4*128 = 512 f32 = one PSUM bank
ROWS_PER_SLAB = 8   # 2 banks per image, 4 banks per slab
N_TAPS = 9
N_JUNK = 36

_cache = {}


def _build_nc():
    import concourse.mybir as mybir
    from concourse import bacc
    from concourse.tile import TileContext

    f32 = mybir.dt.float32
    f16 = mybir.dt.float16

    nc = bacc.Bacc(target_bir_lowering=False)
    # partitions 0-63: img0 padded channels; 64-127: img1
    x_d = nc.dram_tensor("x", [128, HPAD * WPAD], f16, kind="ExternalInput")
    # w[tap] duplicated on both partition halves: wb[p, t*128+co]
    wb_d = nc.dram_tensor("wb", [128, N_TAPS * COUT], f16, kind="ExternalInput")
    b_d = nc.dram_tensor("b", [COUT, 1], f32, kind="ExternalInput")
    # [cout, h, img, w] fp16; host transposes to [img, cout, h, w] + f32
    out_d = nc.dram_tensor(
        "out", [COUT, H * IMGS_PER_CORE * W], f16, kind="ExternalOutput"
    )

    with TileContext(nc) as tc:
        with (
            tc.tile_pool(name="wpool", bufs=1) as wpool,
            tc.tile_pool(name="xpool", bufs=1) as xpool,
            tc.tile_pool(name="opool", bufs=5) as opool,
            tc.tile_pool(name="pspool", bufs=2, space="PSUM") as pspool,
        ):
            wb_sb = wpool.tile([128, N_TAPS * COUT], f16)
            b_f32 = wpool.tile([COUT, 1], f32)
            b_sb = b_f32[:]

            X = xpool.tile([128, HPAD * WPAD], f16)

            # During the 8-core startup crunch each HWDGE ring delivers
            # ~1 packet (= 1 partition's slice) per DMA engine per
            # ~220-300ns, i.e. ~2us per [128, *] DMA instruction,
            # near-independent of byte count -- and the two HW rings
            # (sync, scalar) progress in parallel.  Later x chunks are
            # merged into as few DMAs as the streaming schedule allows
            # to minimize packet rounds.
            junk_src = wpool.tile([128, COUT], f16)
            nc.gpsimd.memset(junk_src[:], 0)
            # The critical transfers go one per ring, whole: wb (8
            # packets/engine) on sync, x rows 0-6 on scalar -- the
            # balanced 8+8 split of the 16 critical packet rounds.
            # x rows 6-17 (needed by slab 0's second bank pair at
            # ~12.8us) is partition-split across both rings so each
            # half is a half round and fires ~12.5us.
            nc.sync.dma_start(out=wb_sb[:], in_=wb_d[:])
            nc.scalar.dma_start(out=X[:, 0 : 6 * WPAD], in_=x_d[:, 0 : 6 * WPAD])
            nc.sync.dma_start(
                out=X[0:64, 6 * WPAD : 18 * WPAD], in_=x_d[0:64, 6 * WPAD : 18 * WPAD]
            )
            nc.scalar.dma_start(
                out=X[64:128, 6 * WPAD : 18 * WPAD],
                in_=x_d[64:128, 6 * WPAD : 18 * WPAD],
            )
            # remaining rows stream in few/large chunks to keep the
            # completion sems ahead of the PE
            for r0, r1 in [(18, 30), (30, 62), (62, 94), (94, HPAD)]:
                nc.sync.dma_start(
                    out=X[:, r0 * WPAD : r1 * WPAD],
                    in_=x_d[:, r0 * WPAD : r1 * WPAD],
                )
            nc.scalar.dma_start(out=b_f32[:], in_=b_d[:])
            X3 = X.rearrange("p (r c) -> p r c", c=WPAD)

            # HAM warm-up: junk matmuls on a memset scratch tile bridge
            # the PE from the engine-sync preamble (~7.3us) to the first
            # input chunk landing (~11.2us), so the activity monitor
            # un-throttles the PE clock (1.2 -> 2.4 GHz) at ~10.7us.
            # Results land in a PSUM bank that slab 0 later overwrites
            # with start=True.
            warm = pspool.tile([COUT, ROWS_PER_BANK * W], f32, tag="psA0")
            for _ in range(N_JUNK):
                nc.tensor.matmul(
                    warm[:, 0:COUT],
                    junk_src[:],
                    junk_src[:],
                    start=True,
                    stop=True,
                )

            # 8-row slabs, then a 4-row and two 2-row slabs at the end:
            # short final bias-adds and a tiny final store reach the
            # teardown barrier ASAP.  The 2-row slabs accumulate in the
            # psA1/psB1 tag slots, whose buffers were freed by slabs
            # 13/14 ~2us before they are needed -- using psA0/psB0
            # would stall the PE on slab 15's bias-add.
            slabs = [(8 * s, 8) for s in range(15)] + [(120, 4), (124, 2), (126, 2)]
            for si, (h0, nrows) in enumerate(slabs):
                h1 = h0 + ROWS_PER_BANK
                bank_rows = min(nrows, ROWS_PER_BANK)
                tagA, tagB = ("psA1", "psB1") if nrows == 2 else ("psA0", "psB0")
                psA0 = pspool.tile([COUT, bank_rows * W], f32, tag=tagA)
                psB0 = pspool.tile([COUT, bank_rows * W], f32, tag=tagB)
                if nrows == 8:
                    psA1 = pspool.tile([COUT, ROWS_PER_BANK * W], f32, tag="psA1")
                    psB1 = pspool.tile([COUT, ROWS_PER_BANK * W], f32, tag="psB1")
                    pairs = [(psA0, psB0, h0), (psA1, psB1, h1)]
                else:
                    pairs = [(psA0, psB0, h0)]
                # slab 0 runs group-major: its first bank pair's nine
                # taps read only input rows 0-5 (the first sync chunk),
                # so compute starts before rows 6-9 land
                order = (
                    [(p, t) for p in pairs for t in range(N_TAPS)]
                    if si == 0
                    else [(p, t) for t in range(N_TAPS) for p in pairs]
                )
                for (psA, psB, h), t in order:
                    kh, kw = divmod(t, 3)
                    lo = wb_sb[0:CIN, t * COUT : (t + 1) * COUT]
                    hi = wb_sb[CIN:128, t * COUT : (t + 1) * COUT]
                    st = t == 0
                    sp = t == N_TAPS - 1
                    # adjacent lo/hi matmuls run concurrently on
                    # disjoint PE row-group halves (different banks)
                    nc.tensor.matmul(
                        psA[:],
                        lo,
                        X3[0:CIN, h + kh : h + kh + bank_rows, kw : kw + W],
                        start=st,
                        stop=sp,
                    )
                    nc.tensor.matmul(
                        psB[:],
                        hi,
                        X3[CIN:128, h + kh : h + kh + bank_rows, kw : kw + W],
                        start=st,
                        stop=sp,
                    )
                # bias-add into fp16 tile, layout [r, img(2), w(128)]
                ob = opool.tile([COUT, nrows * IMGS_PER_CORE * W], f16)
                obv = ob.rearrange("p (r i c) -> p r i c", i=IMGS_PER_CORE, c=W)
                psA0v = psA0.rearrange("p (r c) -> p r c", c=W)
                psB0v = psB0.rearrange("p (r c) -> p r c", c=W)
                out_col = h0 * IMGS_PER_CORE * W
                span = nrows * IMGS_PER_CORE * W
                if nrows == 8:
                    psA1v = psA1.rearrange("p (r c) -> p r c", c=W)
                    psB1v = psB1.rearrange("p (r c) -> p r c", c=W)
                    nc.scalar.add(obv[:, 0:4, 0, :], psA0v[:], b_sb)
                    nc.scalar.add(obv[:, 0:4, 1, :], psB0v[:], b_sb)
                    nc.vector.tensor_scalar_add(obv[:, 4:8, 0, :], psA1v[:], b_sb)
                    nc.vector.tensor_scalar_add(obv[:, 4:8, 1, :], psB1v[:], b_sb)
                    # one contiguous 512 KB store per slab
                    nc.scalar.dma_start(
                        out=out_d[:, out_col : out_col + span],
                        in_=ob[:, 0:span],
                    )
                elif si < len(slabs) - 1:
                    # 4/2-row slab: both engines evacuate in parallel
                    # (different banks); store on the otherwise-idle
                    # sync ring to keep the scalar ring drained for the
                    # final store
                    nc.scalar.add(obv[:, 0:nrows, 0, :], psA0v[:], b_sb)
                    nc.vector.tensor_scalar_add(obv[:, 0:nrows, 1, :], psB0v[:], b_sb)
                    nc.sync.dma_start(
                        out=out_d[:, out_col : out_col + span],
                        in_=ob[:, 0:span],
                    )
                else:
                    # final 2-row slab: one short bias-add per engine,
                    # then the store split by PARTITION halves across
                    # both rings -- 64 descriptors each is half a
                    # DMA-engine round, so the last store's transfer +
                    # HBM receipt (which gate the teardown) finish
                    # ~1us earlier than a 128-partition store would.
                    nc.scalar.add(obv[:, 0:nrows, 0, :], psA0v[:], b_sb)
                    nc.vector.tensor_scalar_add(obv[:, 0:nrows, 1, :], psB0v[:], b_sb)
                    nc.sync.dma_start(
                        out=out_d[0:64, out_col : out_col + span],
                        in_=ob[0:64, 0:span],
                    )
                    nc.scalar.dma_start(
                        out=out_d[64:128, out_col : out_col + span],
                        in_=ob[64:128, 0:span],
                    )
    nc.compile()
    _dedup_ldweights(nc)
    # NOTE: _hoist_startup_dmas (moving the critical input DMAs ahead
    # of the all-engine barrier) measured ~0.5-1us faster but produced
    # nonfinite output on ~half the runs once machine timing drifted --
    # the pre-barrier descriptors race per-core DGE initialization and
    # slab 0 then consumes garbage.  Correctness gates the speedup, so
    # the hoist is disabled.
    return nc


def _hoist_startup_dmas(nc):
    """Move the two startup-critical DMA instructions (wb on sync,
    x rows 0-6 on scalar) from the tile block into the entry block,
    ahead of the all-engine barrier.

    The issuing engines are idle before the barrier (~6.4us) while the
    barrier itself only completes ~7.0us, so hoisting rings the HWDGE
    doorbells ~0.6us earlier and the rings wake sooner.  The hoisted
    DMAs carry no semaphore waits and their completion increments fire
    wherever the instruction lives, so the tile-side consumers are
    unaffected.
    """
    blocks = list(nc.main_func.blocks)
    main = next(bb for bb in blocks if bb.name == "main")
    tile_bb = next(
        bb
        for bb in blocks
        if "tile_context" in bb.name and not bb.name.endswith("_end")
    )
    tinsts = tile_bb.instructions
    limits = {"Activation": 2, "SP": 2}
    hoist = {"Activation": [], "SP": []}
    for inst in tinsts:
        eng = str(inst.engine).split(".")[-1]
        if (
            type(inst).__name__ == "InstDMACopy"
            and eng in hoist
            and len(hoist[eng]) < limits[eng]
        ):
            si = inst.sync_info
            if si and si.on_wait:
                continue
            hoist[eng].append(inst)
    minsts = main.instructions
    for eng, insts in hoist.items():
        for inst in insts:
            tinsts.remove(inst)
        pos = next(
            k
            for k, mi in enumerate(minsts)
            if str(mi.engine).split(".")[-1] == eng
        )
        for off, inst in enumerate(insts):
            minsts.insert(pos + off, inst)

    # The HAM warm-up (junk matmuls) intentionally stays in the tile
    # block: hoisting it pre-barrier was tried and regressed -- the
    # busy PE delays the barrier (starving the later input chunks) and
    # the junk-to-data seam gap then resets the free-running HAM
    # window on ~half the draws, restarting the clock throttle.  With
    # the junk post-barrier it always abuts the data arrival.


def _dedup_ldweights(nc):
    """Drop InstLdweights that reload the exact weights AP already
    resident in the same PE-array half.

    The tap-major slab order issues, per tap, the lo weights for bank
    pair 0 and again for bank pair 1 (same SBUF slice -> same array
    rows); the legalizer emits a fresh InstLdweights for every matmul.
    The second load is a no-op on the array state, but costs an NX
    issue slot (~3ns each, ~0.8us over the kernel).  LDW loads
    alternate between the two array halves (lo = rows 0-63, hi =
    64-127) and a load into one half does not disturb the other, so an
    LDW whose AP equals the AP of the LDW two back in the PE stream is
    reloading exactly what is still resident.  Only waitless/updateless
    LDWs are removed so the semaphore schedule is untouched.
    """
    for bb in nc.main_func.blocks:
        if "tile_context" not in bb.name or bb.name.endswith("_end"):
            continue
        insts = bb.instructions
        keys = []
        to_remove = []
        for inst in insts:
            if type(inst).__name__ != "InstLdweights":
                continue
            si = inst.sync_info
            has_sync = bool(si and (si.on_wait or si.on_update))
            key = str(inst.ins[0])
            if len(keys) >= 2 and keys[-2] == key and not has_sync:
                to_remove.append(inst)
            keys.append(key)
        for inst in to_remove:
            insts.remove(inst)


def _get_nc():
    if "nc" not in _cache:
        _cache["nc"] = _build_nc()
    return _cache["nc"]


def _prepare_in_maps(input_tensor, weights, bias):
    input_tensor = np.asarray(input_tensor, dtype=np.float32)
    weights = np.asarray(weights, dtype=np.float32)
    bias = np.asarray(bias, dtype=np.float32)
    # wb[ci, t*128+co] = W[co, ci, kh, kw], t = kh*3+kw; both halves
    w9 = weights.transpose(1, 2, 3, 0).reshape(CIN, N_TAPS * COUT)  # ci,(kh kw co)
    wb = np.empty((128, N_TAPS * COUT), dtype=np.float16)
    wb[0:CIN] = w9
    wb[CIN:128] = w9
    wb = np.ascontiguousarray(wb)
    b = np.ascontiguousarray(bias.reshape(COUT, 1))
    in_maps = []
    for c in range(N_CORES):
        imgs = input_tensor[c * IMGS_PER_CORE : (c + 1) * IMGS_PER_CORE]
        zp = np.zeros((IMGS_PER_CORE, CIN, HPAD, WPAD), dtype=np.float16)
        zp[:, :, 1 : H + 1, 1 : W + 1] = imgs
        shard = np.ascontiguousarray(zp.reshape(128, HPAD * WPAD))
        in_maps.append({"x": shard, "wb": wb, "b": b})
    return in_maps


def _gather(results):
    outs = []
    for c in range(N_CORES):
        o = results[c]["out"].reshape(COUT, H, IMGS_PER_CORE, W)
        outs.append(np.ascontiguousarray(o.transpose(2, 0, 1, 3), dtype=np.float32))
    return np.concatenate(outs, axis=0)


def kernel(input_tensor, weights, bias):
    from concourse.bass_utils import run_bass_kernel_spmd

    nc = _get_nc()
    in_maps = _prepare_in_maps(input_tensor, weights, bias)
    res = run_bass_kernel_spmd(nc, in_maps, core_ids=list(range(N_CORES)))
    return _gather(res.results)
